# revision 26
# baseline (speedup 1.0000x reference)
"""CNN + truncated path-signature (depth 4) + FF head on 8 TRN2 NeuronCores.

Strategy (v2)
-------------
- Batch data-parallel signature: core c owns batches [8c, 8c+8) = 32
  (batch, out_ch) lanes, T=128 steps on partitions. Levels 3/4 are plain
  T-contractions on PE (suffix-vector trick); the only scan is one
  triangular matmul.
- Level-1/2 features (360 rows of w0) are handled locally: every core
  keeps that w0 slice and adds its own batches' contribution after the
  ReduceScatter, so the AllToAll carries only level-3/4 features.
- zl is packed BATCH-major per shard ([shard, bloc, feat]) so both the
  pack writes (720B lines) and the post-A2A reads are large-line DMAs;
  the feature-major layout the GEMM needs is recovered with one XBAR
  dma_start_transpose (14ns per 16x128 tile).
- w0 is K-sharded by ij-blocks (360 feats each): 10 blocks on cores 0-6,
  11 on core 7, zero-padded to a uniform 4096-row shard; w0 rows are
  permuted host-side to the kernel's (jj, oc, m) order.
- dx/P1/pl(broadcast of p[T-1]) come from single PE matmuls against
  host-built bidiagonal / ones matrices instead of DMA shifts/bounces.
"""
import os
import sys
sys.path.insert(0, "/opt/trn_rl_repo")
if os.environ.get("JAX_PLATFORMS") == "cpu":
    os.environ["JAX_PLATFORMS"] = ""

import numpy as np
import bass_rust as _bass_rust
import concourse.bass as bass
import concourse.tile as tile
import concourse.mybir as mybir
from concourse.vector_clock import ScopedClock
from concourse.bass_utils import run_bass_kernel_spmd

F32 = mybir.dt.float32
F32R = mybir.dt.float32r
BF16 = mybir.dt.bfloat16
AL = mybir.AluOpType
AF = mybir.ActivationFunctionType

NCORES = 8
B, T, IN_CH = 64, 128, 32
OUT_CH, CH, D = 4, 8, 9
BL = B // NCORES                   # local batches = 8
LANES = BL * OUT_CH                # 32 lanes/core
NG = 4                             # lane groups (2 blocs x 4 oc = 8 lanes)
GL = 8
SIGC = 7380
H0, H1, NCLS = 512, 256, 10
W = LANES * D                      # 288

# L34 sharding: ij-blocks of 360 feats (4 oc x 90 m); 81 blocks total.
NIJ = [10] * 7 + [11]
IJ0 = [0, 10, 20, 30, 40, 50, 60, 70]
CHUNK = 4096                       # padded feats per shard
KT = CHUNK // 128                  # 32 k-tiles


class _SplitDrainTileContext(tile.TileContext):
    """Tile exit drain carries one sem wait per CTRL instruction.

    This container's walrus build rejects >2 sync waits on a CTRL
    instruction; Tile's exit drain waits on the whole global clock.
    Redistribute the waits over nops on the same engine (program order on
    one engine preserves semantics)."""

    MAX_WAITS = 1

    def _split_body_waits(self):
        nc = self.nc
        for bb in nc.main_func.blocks:
            heavy = [ins for ins in bb.instructions
                     if ins.sync_info and ins.sync_info.on_wait
                     and len(ins.sync_info.on_wait) > self.MAX_WAITS]
            if not heavy:
                continue
            extra = {}
            for ins in heavy:
                w = list(ins.sync_info.on_wait)
                ins.sync_info.on_wait = w[:self.MAX_WAITS]
                nops = []
                for i in range(self.MAX_WAITS, len(w), self.MAX_WAITS):
                    n = nc.engines[ins.engine].nop(hint="wait_split")
                    for bb2 in nc.main_func.blocks:
                        if bb2.instructions and bb2.instructions[-1] is n.ins:
                            bb2.instructions.pop()
                            break
                    for wt in w[i:i + self.MAX_WAITS]:
                        handle = _bass_rust.SemaphoreHandle(wt.ant_name, wt.id)
                        _bass_rust.wait_op(n.ins, handle, wt.wait_value,
                                           "sem-ge", False)
                    nops.append(n.ins)
                extra[id(ins)] = nops
            new_list = []
            for ins in bb.instructions:
                new_list.extend(extra.get(id(ins), ()))
                new_list.append(ins)
            bb.instructions[:] = new_list

    def _drain_and_barrier(self, tick_clock, wait_clock):
        nc = self.nc
        self._split_body_waits()
        probe = nc.sync.nop(hint="tile_exit_wait_0")
        wait_clock.add_sem_waits(
            probe.ins, ScopedClock({None: tick_clock.global_clock})
        )
        waits = list(probe.ins.sync_info.on_wait or [])
        probe.ins.sync_info.on_wait = waits[:1]
        for w in waits[1:]:
            n = nc.sync.nop(hint="tile_exit_wait")
            handle = _bass_rust.SemaphoreHandle(w.ant_name, w.id)
            _bass_rust.wait_op(n.ins, handle, w.wait_value, "sem-ge", False)
        nc.sync.drain()
        nc.all_engine_barrier()
        assert self.sems is not None
        popped = nc._tile_sem_poison_stack.pop()
        assert popped is self._sem_poison
        nc.clear_and_free_semaphores(list(self.sems.allocated().values()))
        nc.all_engine_barrier()


def _ap(t, extra, *dims):
    base = t[:]
    return bass.AP(base.tensor, base.offset + extra, list(dims))


def _build():
    nc = bass.Bass(num_devices=NCORES, target_bir_lowering=True, trn_type="TRN2")

    xs = nc.dram_tensor("xs", [T, BL, IN_CH], F32, kind="ExternalInput")
    cwr = nc.dram_tensor("cwr", [128, 16], F32, kind="ExternalInput")
    cbr = nc.dram_tensor("cbr", [128, OUT_CH], F32, kind="ExternalInput")
    tlin = nc.dram_tensor("tlin", [128, 1], F32, kind="ExternalInput")
    ltri = nc.dram_tensor("ltri", [128, 128], BF16, kind="ExternalInput")
    pmats = nc.dram_tensor("pmats", [128, 6 * 128], F32,
                           kind="ExternalInput")
    oner = nc.dram_tensor("oner", [128, 2], BF16, kind="ExternalInput")
    idn = nc.dram_tensor("idn", [128, 128], F32, kind="ExternalInput")
    w0s = nc.dram_tensor("w0s", [CHUNK, H0], BF16, kind="ExternalInput")
    w12a = nc.dram_tensor("w12a", [9, OUT_CH * H0], BF16, kind="ExternalInput")
    w12b = nc.dram_tensor("w12b", [81, OUT_CH * H0], BF16, kind="ExternalInput")
    w1s = nc.dram_tensor("w1s", [H0, H1], F32, kind="ExternalInput")
    w2s = nc.dram_tensor("w2s", [H1, NCLS], F32, kind="ExternalInput")
    b0c = nc.dram_tensor("b0c", [H0, 1], F32, kind="ExternalInput")
    b1c = nc.dram_tensor("b1c", [H1, 1], F32, kind="ExternalInput")
    b2r = nc.dram_tensor("b2r", [BL, NCLS], F32, kind="ExternalInput")
    out = nc.dram_tensor("out", [BL, NCLS], F32, kind="ExternalOutput")

    with _SplitDrainTileContext(nc) as tc:
        with tc.tile_pool(name="dram", bufs=1, space="DRAM") as dram:
            zl = dram.tile([B, CHUNK], BF16)              # [shard*8+bloc, feat]
            zex = dram.tile([B, CHUNK], BF16)
            cin = dram.tile([B, H0], F32)
            cout = dram.tile([BL, H0], F32)
            prow = dram.tile([1, W], F32)

            with tc.tile_pool(name="const", bufs=1) as cpool, \
                 tc.tile_pool(name="w0p", bufs=1) as w0pool, \
                 tc.tile_pool(name="prep", bufs=1) as ppool:
                # ---- critical-path loads on scalar queue ----
                xs_sb = cpool.tile([128, BL * IN_CH], F32)
                nc.sync.dma_start(xs_sb[:], _ap(xs, 0, [BL * IN_CH, 128],
                                                [1, BL * IN_CH]))
                cw_sb = cpool.tile([128, 16], F32)
                nc.scalar.dma_start(cw_sb[:], cwr[:])
                cb_sb = cpool.tile([128, OUT_CH], F32)
                nc.scalar.dma_start(cb_sb[:], cbr[:])
                tl_sb = cpool.tile([128, 1], F32)
                nc.scalar.dma_start(tl_sb[:], tlin[:])
                pm_sb = cpool.tile([128, 6 * 128], F32)
                nc.scalar.dma_start(pm_sb[:], pmats[:])
                lt_sb = cpool.tile([128, 128], BF16)
                nc.scalar.dma_start(lt_sb[:], ltri[:])
                oner_sb = cpool.tile([128, 2], BF16)
                nc.scalar.dma_start(oner_sb[:], oner[:])
                # ---- w0 shard tiles: ALL issued after the group loop
                # (the A2A window) -- bulk DMA concurrent with compute
                # starves the compute engines on this part.
                w0_t = [w0pool.tile([128, H0], BF16, tag=f"w0_{i}",
                                    name=f"w0_{i}") for i in range(KT)]
                for i in range(4):
                    nc.gpsimd.dma_start(w0_t[i][:],
                                        w0s[i * 128:(i + 1) * 128, :])
                # tail-phase tiles (DMAs emitted after the group loop)
                idn_sb = cpool.tile([128, 128], F32)
                w12a_sb = cpool.tile([9, OUT_CH * H0], BF16)
                w12b_sb = cpool.tile([81, OUT_CH * H0], BF16)
                w1_sb = [cpool.tile([128, H1], F32, tag=f"w1_{j}",
                                    name=f"w1_{j}") for j in range(4)]
                w2_sb = [cpool.tile([128, NCLS], F32, tag=f"w2_{j}",
                                    name=f"w2_{j}") for j in range(2)]
                b0_sb = cpool.tile([128, 4], F32)
                b1_sb = cpool.tile([128, 2], F32)
                b2_sb = cpool.tile([BL, NCLS], F32)

                # gpsimd ucode warm-up (first-use of each op costs ~2us)
                gw = ppool.tile([1, 8], F32, tag="gw")
                nc.gpsimd.memset(gw[:], 1.0)
                nc.gpsimd.tensor_tensor(gw[0:1, 0:2], gw[0:1, 2:4],
                                        gw[0:1, 4:6], AL.mult)
                nc.gpsimd.tensor_scalar(gw[0:1, 0:2], gw[0:1, 2:4], 0.5, None,
                                        AL.mult)
                nc.gpsimd.tensor_copy(gw[0:1, 0:2], gw[0:1, 2:4])

                # ---- zero the zl pad (regions no pack DMA ever writes) ----
                zpad = ppool.tile([B, CHUNK - 3600], BF16, tag="zpad")
                nc.gpsimd.memset(zpad[:], 0.0)
                nc.gpsimd.dma_start(
                    _ap(zl, 3960, [CHUNK, B], [1, CHUNK - 3960]),
                    zpad[:, 0:CHUNK - 3960])
                nc.gpsimd.dma_start(
                    _ap(zl, 3600, [CHUNK, 7 * BL], [1, 360]),
                    zpad[0:7 * BL, 0:360])

                # ---- prep: conv -> path p; dx/P1/pl via PE matmuls ----
                p = ppool.tile([128, W], F32, tag="p")
                dx = ppool.tile([128, W], BF16, tag="dx")
                yt = ppool.tile([128, W], BF16, tag="yt")
                ut = ppool.tile([128, W], BF16, tag="ut")
                u2 = ppool.tile([128, W], BF16, tag="u2")
                at = ppool.tile([128, W], BF16, tag="at")
                dxh = ppool.tile([128, W], BF16, tag="dxh")
                tmpa = ppool.tile([128, BL * IN_CH], F32, tag="tmpa")
                tmpb = ppool.tile([128, BL * IN_CH], F32, tag="tmpb")
                tmpc = ppool.tile([128, BL * IN_CH], F32, tag="tmpc")

                pdst = _ap(p, 1, [W, 128], [D, OUT_CH], [4 * D, BL], [1, CH])
                tva = _ap(tmpa, 0, [BL * IN_CH, 128],
                          [CH * BL, OUT_CH], [CH, BL], [1, CH])
                tvb = _ap(tmpb, 0, [BL * IN_CH, 128],
                          [CH * BL, OUT_CH], [CH, BL], [1, CH])
                tvc = _ap(tmpc, 0, [BL * IN_CH, 128],
                          [CH * BL, OUT_CH], [CH, BL], [1, CH])

                def xsv(k):
                    return _ap(xs_sb, k, [BL * IN_CH, 128],
                               [0, OUT_CH], [IN_CH, BL], [4, CH])

                def cwv(k):
                    return _ap(cw_sb, k, [16, 128], [4, OUT_CH], [0, BL],
                               [0, CH])

                # conv: k0,k2 on vector; k1,k3 on gpsimd; tree add
                nc.vector.tensor_tensor(pdst, xsv(0), cwv(0), AL.mult)
                nc.gpsimd.tensor_tensor(tva, xsv(1), cwv(1), AL.mult)
                nc.vector.tensor_tensor(tvb, xsv(2), cwv(2), AL.mult)
                nc.vector.tensor_tensor(pdst, pdst, tvb, AL.add)
                nc.gpsimd.tensor_tensor(tvc, xsv(3), cwv(3), AL.mult)
                nc.gpsimd.tensor_tensor(tva, tva, tvc, AL.add)
                nc.vector.tensor_tensor(pdst, pdst, tva, AL.add)
                cbv = _ap(cb_sb, 0, [OUT_CH, 128], [1, OUT_CH], [0, BL],
                          [0, CH])
                nc.vector.tensor_tensor(pdst, pdst, cbv, AL.add)
                nc.vector.tensor_copy(_ap(p, 0, [W, 128], [D, LANES]),
                                      _ap(tl_sb, 0, [1, 128], [0, LANES]))

                # dx/yt/ut/u2/at/dxh are all <const 128x128> @ p: six PE
                # matmuls + cheap PSUM evacs (elementwise prep ops measure
                # 6-20x slow here when DVE and Pool run concurrently).
                with tc.tile_pool(name="pprep", bufs=1, space="PSUM") as pprep:
                    outs = [(dx, 0), (yt, 1), (ut, 0), (u2, 1), (at, 0),
                            (dxh, 1)]
                    for i, (dst, which) in enumerate(outs):
                        pp = pprep.tile([128, W], F32, tag=f"pp{i}",
                                        name=f"pp{i}")
                        nc.tensor.matmul(pp[:],
                                         pm_sb[:, i * 128:(i + 1) * 128],
                                         p[:], start=True, stop=True)
                        if which == 0:
                            nc.vector.tensor_copy(dst[:], pp[:])
                        else:
                            nc.scalar.activation(dst[:], pp[:], AF.Copy)
                # S1 bounce (off critical path)
                nc.scalar.dma_start(prow[:], p[127:128, :])
                z1tmp = ppool.tile([9, LANES], F32, tag="z1tmp")
                nc.scalar.dma_start(z1tmp[:], _ap(prow, 0, [1, 9], [D, LANES]))

                # local L1/L2 features, transposed: [oc*8+bloc] free cols
                zs1 = cpool.tile([9, LANES], BF16, tag="zs1", name="zs1")
                zs2 = cpool.tile([81, LANES], BF16, tag="zs2", name="zs2")

                # ---- signature groups ----
                with tc.tile_pool(name="grp", bufs=2) as gpool, \
                     tc.tile_pool(name="ps2", bufs=2, space="PSUM") as ps2p, \
                     tc.tile_pool(name="ptab", bufs=1, space="PSUM") as ptab, \
                     tc.tile_pool(name="ps12", bufs=1, space="PSUM") as ps12p:
                    for g in range(NG):
                        off = g * GL * D
                        GW = GL * 81      # 648

                        def o_ij(t, st=1):
                            return _ap(t, off, [W, 128], [D, GL], [st, D],
                                       [0, D])

                        def o_ji(t, st=1):
                            return _ap(t, off, [W, 128], [D, GL], [0, D],
                                       [st, D])

                        m2 = gpool.tile([128, GW], BF16, tag="m2")
                        m2v = _ap(m2, 0, [GW, 128], [81, GL], [D, D], [1, D])
                        nc.vector.tensor_tensor(m2v, o_ij(at), o_ji(dx),
                                                AL.mult)

                        s2 = ps2p.tile([128, GW], F32, tag="s2")
                        nc.tensor.matmul(s2[:, 0:512], lt_sb[:], m2[:, 0:512],
                                         start=True, stop=True)
                        nc.tensor.matmul(s2[:, 512:GW], lt_sb[:],
                                         m2[:, 512:GW], start=True, stop=True)

                        bt = gpool.tile([128, GW], BF16, tag="bt")
                        btv = _ap(bt, 0, [GW, 128], [81, GL], [D, D], [1, D])
                        nc.vector.tensor_tensor(btv, o_ij(ut), o_ji(dx),
                                                AL.mult)
                        nc.vector.tensor_tensor(bt[:], bt[:], s2[:], AL.add)

                        t8 = gpool.tile([128, GW], BF16, tag="t8")
                        t8v = _ap(t8, 0, [GW, 128], [81, GL], [D, D], [1, D])
                        nc.vector.tensor_tensor(t8v, o_ij(u2), o_ji(dx),
                                                AL.mult)
                        nc.vector.tensor_tensor(t8[:], t8[:], s2[:], AL.add)

                        rx = gpool.tile([128, GL * 90], BF16, tag="rx")
                        rxv = _ap(rx, 0, [GL * 90, 128], [90, GL], [D, D],
                                  [1, D])
                        nc.gpsimd.tensor_tensor(rxv, o_ij(dx), o_ji(yt),
                                                AL.mult)
                        nc.gpsimd.tensor_copy(
                            _ap(rx, 81, [GL * 90, 128], [90, GL], [1, D]),
                            _ap(dx, off, [W, 128], [D, GL], [1, D]))

                        q2 = gpool.tile([128, GL * 82], BF16, tag="q2")
                        q2v = _ap(q2, 0, [GL * 82, 128], [82, GL], [D, D],
                                  [1, D])
                        nc.gpsimd.tensor_tensor(q2v, o_ij(dxh), o_ji(dx),
                                                AL.mult)
                        nc.vector.tensor_scalar(
                            _ap(q2, 81, [GL * 82, 128], [82, GL]),
                            _ap(dxh, 0, [W, 128], [0, GL]),
                            0.0, None, AL.mult)

                        tab = ptab.tile([128, GL * 90], F32, tag="tab")
                        s12 = ps12p.tile([128, 2 * GL], F32, tag="s12")
                        for l in range(GL):
                            nc.tensor.matmul(
                                _ap(tab, 90 * l, [GL * 90, 81], [1, 90]),
                                bt[:, l * 81:(l + 1) * 81],
                                rx[:, l * 90:(l + 1) * 90],
                                start=True, stop=False)
                            nc.tensor.matmul(s12[0:81, 2 * l:2 * l + 2],
                                             m2[:, l * 81:(l + 1) * 81],
                                             oner_sb[:], start=True, stop=True)
                            nc.tensor.matmul(
                                _ap(tab, 90 * l, [GL * 90, 81], [1, 82]),
                                t8[:, l * 81:(l + 1) * 81],
                                q2[:, l * 82:(l + 1) * 82],
                                start=False, stop=True)

                        # evac: both halves are contiguous 360-el runs
                        zt4g = gpool.tile([81, 2 * 360], BF16, tag="zt4g")
                        nc.scalar.activation(
                            _ap(zt4g, 0, [720, 81], [1, 360]),
                            _ap(tab, 0, [GL * 90, 81], [1, 360]),
                            AF.Copy)
                        nc.scalar.activation(
                            _ap(zt4g, 360, [720, 81], [1, 360]),
                            _ap(tab, 360, [GL * 90, 81], [1, 360]),
                            AF.Copy)
                        # S2 -> zT12[9+ij, oc*8 + bloc]
                        nc.vector.tensor_copy(
                            _ap(zs2, 2 * g, [LANES, 81], [8, OUT_CH], [1, 2]),
                            _ap(s12, 0, [2 * GL, 81], [2, OUT_CH], [8, 2]),
                        )

                        # pack into zl: per bloc-parity, shards 0-6 + shard 7
                        for bo in range(2):
                            nc.sync.dma_start(
                                _ap(zl, (2 * g + bo) * CHUNK,
                                    [8 * CHUNK, 7], [360, 10], [1, 360]),
                                _ap(zt4g, bo * 360, [720, 70], [1, 360]))
                            nc.sync.dma_start(
                                _ap(zl, (7 * 8 + 2 * g + bo) * CHUNK,
                                    [360, 11], [1, 360]),
                                _ap(zt4g, 70 * 720 + bo * 360, [720, 11],
                                    [1, 360]))

                # S1 -> zT12[0:9, oc*8 + bloc]
                nc.vector.tensor_copy(
                    _ap(zs1, 0, [LANES, 9], [8, OUT_CH], [1, BL]),
                    _ap(z1tmp, 0, [LANES, 9], [1, OUT_CH], [OUT_CH, BL]))

                # w0 stream + tail consts in the A2A window
                for i in range(2, KT // 2):
                    nc.sync.dma_start(w0_t[2 * i][:],
                                      w0s[2 * i * 128:(2 * i + 1) * 128, :])
                    nc.scalar.dma_start(
                        w0_t[2 * i + 1][:],
                        w0s[(2 * i + 1) * 128:(2 * i + 2) * 128, :])
                nc.scalar.dma_start(idn_sb[:], idn[:])
                nc.scalar.dma_start(w12a_sb[:], w12a[:])
                nc.scalar.dma_start(w12b_sb[:], w12b[:])
                for j in range(4):
                    nc.scalar.dma_start(w1_sb[j][:],
                                        w1s[j * 128:(j + 1) * 128, :])
                for j in range(2):
                    nc.scalar.dma_start(w2_sb[j][:],
                                        w2s[j * 128:(j + 1) * 128, :])
                nc.scalar.dma_start(b0_sb[:], _ap(b0c, 0, [1, 128], [128, 4]))
                nc.scalar.dma_start(b1_sb[:], _ap(b1c, 0, [1, 128], [128, 2]))
                nc.scalar.dma_start(b2_sb[:], b2r[:])

                # ---- AllToAll: zl [64, 4096] -> zex (batch-major shards) ----
                nc.gpsimd.collective_compute(
                    "AllToAll", AL.bypass,
                    replica_groups=[list(range(NCORES))],
                    ins=[zl[:].opt()], outs=[zex[:].opt()])

                with tc.tile_pool(name="gemm", bufs=1) as gepool, \
                     tc.tile_pool(name="pz0", bufs=1, space="PSUM") as pz0p, \
                     tc.tile_pool(name="ptail", bufs=1, space="PSUM") as ptail:
                    # L12 partials into the tail PSUM during the A2A
                    pz1 = ptail.tile([128, 4 * BL], F32, tag="pz1")
                    for oc in range(OUT_CH):
                        for j in range(4):
                            nc.tensor.matmul(
                                pz1[:, j * BL:(j + 1) * BL],
                                _ap(w12b_sb, oc * H0 + j * 128,
                                    [OUT_CH * H0, 81], [1, 128]),
                                _ap(zs2, oc * BL, [LANES, 81], [1, BL]),
                                start=(oc == 0), stop=False)
                            nc.tensor.matmul(
                                pz1[:, j * BL:(j + 1) * BL],
                                _ap(w12a_sb, oc * H0 + j * 128,
                                    [OUT_CH * H0, 9], [1, 128]),
                                _ap(zs1, oc * BL, [LANES, 9], [1, BL]),
                                start=False, stop=False)

                    # XBAR transpose: zex [64, 4096] -> ztlT [128, 32*64]
                    ztlT = gepool.tile([128, KT * B], BF16, tag="ztlT")
                    for h in range(2):
                        nc.sync.dma_start_transpose(
                            bass.AP(ztlT[:].tensor,
                                    ztlT[:].offset + h * 16 * B,
                                    [[KT * B, 128], [B, 16], [1, B]]),
                            _ap(zex, h * 2048, [CHUNK, B], [1, 2048]))

                    # z0 = z^T-tiles @ w0-tiles, PSUM accumulate
                    z0p = pz0p.tile([B, H0], F32, tag="z0p")
                    for i in range(KT):
                        nc.tensor.matmul(z0p[:],
                                         ztlT[:, i * B:(i + 1) * B],
                                         w0_t[i][:],
                                         start=(i == 0), stop=(i == KT - 1))
                    z0sb = gepool.tile([B, H0], F32, tag="z0sb")
                    nc.vector.tensor_copy(z0sb[:], z0p[:])
                    nc.gpsimd.dma_start(cin[:], z0sb[:])
                    nc.gpsimd.collective_compute(
                        "ReduceScatter", AL.add,
                        replica_groups=[list(range(NCORES))],
                        ins=[cin[:].opt()], outs=[cout[:].opt()])

                    # ---- tail ----
                    z1row = gepool.tile([BL, H0], F32, tag="z1row")
                    nc.gpsimd.dma_start(z1row[:], cout[:])
                    z1t = gepool.tile([128, 4 * BL], F32, tag="z1t")
                    for j in range(4):
                        # accumulate z1row^T onto the L12 partials
                        nc.tensor.matmul(pz1[:, j * BL:(j + 1) * BL],
                                         z1row[:, j * 128:(j + 1) * 128],
                                         idn_sb[0:BL, 0:BL],
                                         start=False, stop=True)
                        nc.scalar.activation(z1t[:, j * BL:(j + 1) * BL],
                                             pz1[:, j * BL:(j + 1) * BL],
                                             AF.Sigmoid, bias=b0_sb[:, j:j + 1])
                    pz2 = ptail.tile([128, 2 * BL], F32, tag="pz2")
                    z2t = gepool.tile([128, 2 * BL], F32, tag="z2t")
                    for m in range(2):
                        for kj in range(4):
                            nc.tensor.matmul(
                                pz2[:, m * BL:(m + 1) * BL],
                                w1_sb[kj][:, m * 128:(m + 1) * 128],
                                z1t[:, kj * BL:(kj + 1) * BL],
                                start=(kj == 0), stop=(kj == 3))
                        nc.scalar.activation(z2t[:, m * BL:(m + 1) * BL],
                                             pz2[:, m * BL:(m + 1) * BL],
                                             AF.Sigmoid, bias=b1_sb[:, m:m + 1])
                    dume = gepool.tile([1, 1], F32, tag="dume")
                    nc.scalar.activation(dume[0:1, 0:1], b1_sb[0:1, 0:1],
                                         AF.Exp)
                    pz3 = ptail.tile([BL, NCLS], F32, tag="pz3")
                    for m in range(2):
                        nc.tensor.matmul(pz3[:], z2t[:, m * BL:(m + 1) * BL],
                                         w2_sb[m][:], start=(m == 0),
                                         stop=(m == 1))
                    z3 = gepool.tile([BL, NCLS], F32, tag="z3")
                    nc.vector.tensor_tensor(z3[:], pz3[:], b2_sb[:], AL.add)
                    mx = gepool.tile([BL, 1], F32, tag="mx")
                    nc.vector.tensor_reduce(mx[:], z3[:], mybir.AxisListType.X,
                                            AL.max)
                    tm = gepool.tile([BL, NCLS], F32, tag="tm")
                    nc.vector.tensor_scalar(tm[:], z3[:], mx[:, 0:1], None,
                                            AL.subtract)
                    ex = gepool.tile([BL, NCLS], F32, tag="ex")
                    se = gepool.tile([BL, 1], F32, tag="se")
                    nc.scalar.activation(ex[:], tm[:], AF.Exp, accum_out=se[:])
                    ls = gepool.tile([BL, 1], F32, tag="ls")
                    nc.scalar.activation(ls[:], se[:], AF.Ln)
                    osb = gepool.tile([BL, NCLS], F32, tag="osb")
                    nc.vector.tensor_scalar(osb[:], tm[:], ls[:, 0:1], None,
                                            AL.subtract)
                    nc.gpsimd.dma_start(out[:], osb[:])
    return nc


def _w0_shards(w0):
    """Per-core padded K-shards [CHUNK, H0] in (jj, oc, m) order, plus the
    local L1/L2 slice [90, 4*H0]."""
    bf16 = mybir.dt.np(BF16)
    shards = []
    for c in range(NCORES):
        rows = np.zeros((CHUNK, H0), np.float32)
        nij = NIJ[c]
        jj = np.arange(nij)
        ij = IJ0[c] + jj
        # orig row for (oc, ij, m): m<81 -> L4 819+ij*81+m ; m>=81 -> L3 90+ij*9+(m-81)
        m = np.arange(90)
        orig_m = np.where(m < 81, 819 + m, 90 + (m - 81))  # partial; needs ij scale
        idx = np.empty((nij, OUT_CH, 90), np.int64)
        for och in range(OUT_CH):
            l4 = 819 + ij[:, None] * 81 + np.arange(81)[None, :]
            l3 = 90 + ij[:, None] * 9 + np.arange(9)[None, :]
            idx[:, och, :81] = och * SIGC + l4
            idx[:, och, 81:] = och * SIGC + l3
        rows[:nij * 360] = w0[idx.reshape(-1)]
        shards.append(np.ascontiguousarray(rows.astype(bf16)))
    w12 = np.concatenate([w0[och * SIGC:och * SIGC + 90] for och in
                          range(OUT_CH)], axis=1).astype(bf16)
    return (shards, np.ascontiguousarray(w12[:9]),
            np.ascontiguousarray(w12[9:]))


_CACHE = {}


def kernel(x, conv_w, conv_b, w0, b0, w1, b1, w2, b2):
    x = np.ascontiguousarray(np.asarray(x, np.float32))
    conv_w = np.asarray(conv_w, np.float32)
    conv_b = np.asarray(conv_b, np.float32)
    w0 = np.asarray(w0, np.float32)
    w1 = np.ascontiguousarray(np.asarray(w1, np.float32))
    w2 = np.ascontiguousarray(np.asarray(w2, np.float32))
    b0 = np.asarray(b0, np.float32)
    b1 = np.asarray(b1, np.float32)
    b2 = np.asarray(b2, np.float32)

    if "nc" not in _CACHE:
        _CACHE["nc"] = _build()
    nc = _CACHE["nc"]
    shards, w12a_h, w12b_h = _w0_shards(w0)

    ey = np.eye(128, dtype=np.float32)
    sh = np.eye(128, k=1, dtype=np.float32)   # sh[s,t]=1 iff t=s+1
    mm = ey - sh                              # dx
    ym = -ey.copy()
    ym[127, :] += 1.0                         # yt
    pmats = np.concatenate([
        mm,                                   # dx
        ym,                                   # yt
        ey / 6 + sh / 3,                      # ut = P1/2 + dx/6
        ey / 12 + sh / 4,                     # u2 = P1/3 + dx/12
        (ey + sh) / 2,                        # at = P1 + dx/2
        mm / 2,                               # dxh
    ], axis=1)
    shared = {
        "cwr": np.ascontiguousarray(
            np.broadcast_to(conv_w.reshape(1, 16), (128, 16))),
        "cbr": np.ascontiguousarray(
            np.broadcast_to(conv_b.reshape(1, OUT_CH), (128, OUT_CH))),
        "tlin": np.linspace(0.0, 1.0, T, dtype=np.float32).reshape(128, 1),
        "ltri": np.ascontiguousarray(
            np.triu(np.ones((128, 128), np.float32), 1)).astype(
                mybir.dt.np(BF16)),
        "pmats": np.ascontiguousarray(pmats),
        "oner": np.ones((128, 2), mybir.dt.np(BF16)),
        "idn": np.eye(128, dtype=np.float32),
        "w12a": w12a_h, "w12b": w12b_h,
        "w1s": w1, "w2s": w2,
        "b0c": b0.reshape(H0, 1), "b1c": b1.reshape(H1, 1),
        "b2r": np.ascontiguousarray(np.broadcast_to(b2.reshape(1, NCLS),
                                                    (BL, NCLS))),
    }
    in_maps = []
    for c in range(NCORES):
        m = dict(shared)
        m["xs"] = np.ascontiguousarray(
            x[c * BL:(c + 1) * BL, 0].transpose(1, 0, 2))
        m["w0s"] = shards[c]
        in_maps.append(m)

    _CACHE["in_maps"] = in_maps
    res = run_bass_kernel_spmd(nc, in_maps, core_ids=list(range(NCORES)))
    return np.concatenate([res.results[c]["out"] for c in range(NCORES)],
                          axis=0)
